# revision 14
# baseline (speedup 1.0000x reference)
"""CompressedLinear (int8 weight, per-row scale) on 8 Trainium2 NeuronCores.

Math: y[b,s,o] = sum_i x[b,s,i] * (w_int8[o,i] * scale[o]) + bias[o]

Strategy (tensor-parallel over out_features, per sharding hint):
  - Shard W/scale/bias rows across 8 cores (1376 rows each); x replicated.
  - Scale is applied to the matmul OUTPUT (algebraically identical), so the
    device matmuls run on the raw int8 weights cast to fp16 (int8 is exact
    in fp16); casting x to fp16 bounds the output relative error at ~2e-4.
  - All dtype conversion happens on the HOST (free w.r.t. HW exec time), so
    every device DMA is a plain same-dtype HWDGE transfer from a contiguous
    HBM block — no SWDGE software-descriptor path anywhere.
  - Per-core layout (built host-side):
      xt [4, 8, 128, 2048] fp16 : chunk c, k-group g (4 k-slices of 128),
                                  partition, 4*512 s-columns
      wt [32, 128, 1376]   fp16 : k-slice kt, partition, out-rows
      yt [4, 1376, 512]    fp32 : chunk c, out-rows, s-columns
  - Chunk 0 runs kt-outer over an 8-o-tile PSUM group (DMA delivery rate
    bounds the sweep; 8 tiles amortize each w/x tile over 8 matmuls), then
    the 3 remaining o-tiles.  Chunks 1-3 run ot-outer (x fully prefetched,
    drains spread evenly, minimal PSUM pressure).
  - Per-partition affine (scale, bias) is fused into the PSUM eviction.
"""

import numpy as np

import concourse.bass as bass
import concourse.tile as tile
from concourse import bacc, mybir
from concourse.bass_utils import run_bass_kernel_spmd

B = 1
S = 2048
I = 4096
O = 11008
N_CORES = 8
O_SHARD = O // N_CORES  # 1376
P = 128
SC = 512                # s-columns per matmul (one PSUM bank of fp32)
N_CHUNKS = S // SC      # 4
KT = I // P             # 32 k-slices
XG = 4                  # k-slices per x DMA group
NXG = KT // XG          # 8 x groups per chunk
OT = (O_SHARD + P - 1) // P  # 11 o-tiles (10 full + one of 96 rows)


def build_bass():
    MM_DT = mybir.dt.float16
    nc = bacc.Bacc("TRN2", target_bir_lowering=False, debug=False)

    xt = nc.dram_tensor("xt", [N_CHUNKS, NXG, P, XG * SC], MM_DT,
                        kind="ExternalInput").ap()
    wt = nc.dram_tensor("wt", [KT, P, O_SHARD], MM_DT,
                        kind="ExternalInput").ap()
    # scale/bias pre-rearranged on host to [p, t] = value for o = t*128 + p
    scale = nc.dram_tensor("scale", [P, OT], mybir.dt.float32,
                           kind="ExternalInput").ap()
    bias = nc.dram_tensor("bias", [P, OT], mybir.dt.float32,
                          kind="ExternalInput").ap()
    yt = nc.dram_tensor("yt", [N_CHUNKS, O_SHARD, SC], mybir.dt.float32,
                        kind="ExternalOutput").ap()

    with tile.TileContext(nc) as tc:
        with (
            tc.tile_pool(name="wres", bufs=1) as wres_pool,
            tc.tile_pool(name="consts", bufs=1) as const_pool,
            tc.tile_pool(name="xpool", bufs=16) as xpool,
            tc.tile_pool(name="outp", bufs=4) as out_pool,
            tc.tile_pool(name="psum", bufs=8, space="PSUM") as psum_pool,
        ):
            w_res = [None] * KT
            w_dmas = [None] * KT
            x_tiles = {}

            def emit_w(kt):
                w_kt = wres_pool.tile([P, O_SHARD], MM_DT, tag=f"w{kt}")
                w_dmas[kt] = nc.sync.dma_start(w_kt[:], wt[kt])
                w_res[kt] = w_kt

            def emit_xg(c, g, after_w=None, split=False):
                t = xpool.tile([P, XG * SC], MM_DT, tag="xg")
                if split:
                    # two half-DMAs: the first two k-slices land a transfer
                    # earlier, so the first real matmuls start sooner
                    h = XG * SC // 2
                    xds = [
                        nc.scalar.dma_start(t[:, :h], xt[c, g, :, :h]),
                        nc.scalar.dma_start(t[:, h:], xt[c, g, :, h:]),
                    ]
                else:
                    xds = [nc.scalar.dma_start(t[:], xt[c, g])]
                if after_w is not None:
                    # The per-core HBM/DMA bandwidth is shared across queues;
                    # hold prefetches back so the startup-critical weight
                    # stream is never starved.
                    for xd in xds:
                        bass._add_dep_helper(
                            xd.ins, w_dmas[after_w].ins, sync=True,
                            reason="pace x prefetch behind startup w stream",
                        )
                x_tiles[(c, g)] = t

            # First weight slice + first x group ride at the head of their
            # queues so the first real matmul's inputs land ASAP.
            emit_w(0)
            emit_xg(0, 0, split=True)

            # PE warm-up: dependency-free matmuls keep the PE busy during
            # the initial DMA window so the HAM clock gate opens (K=8/8)
            # before the first real matmul issues.
            warm_sb = const_pool.tile([P, P], MM_DT)
            nc.any.memset(warm_sb[:], 0.0)
            warm_ps = psum_pool.tile([P, P], mybir.dt.float32,
                                     name="warm_ps", tag="psum")
            # Sized so warm-up (at half clock until the HAM gate opens ~4us
            # in) ends right as the first w/x tiles land (~11.9us).
            N_WARM = 42
            for i in range(N_WARM):
                nc.tensor.matmul(
                    warm_ps[:], warm_sb[:], warm_sb[:],
                    start=(i == 0), stop=(i == N_WARM - 1),
                )

            # per-partition scale/bias columns, host-rearranged; gpsimd queue
            # keeps them entirely off the startup-critical sync/scalar queues
            # (not needed until the first PSUM drain).
            scale_t = const_pool.tile([P, OT], mybir.dt.float32)
            bias_t = const_pool.tile([P, OT], mybir.dt.float32)
            nc.gpsimd.dma_start(scale_t[:], scale[:, :])
            nc.gpsimd.dma_start(bias_t[:], bias[:, :])

            # Remaining weights (sync queue) and chunk-0 x groups (scalar
            # queue).  x group g is consumed at kt=4g; pacing it behind
            # w[4g-1] keeps the shared DMA bandwidth on the weight stream.
            for kt in range(1, KT):
                emit_w(kt)
            for g in range(1, NXG):
                emit_xg(0, g, after_w=max(1, 4 * g - 3), split=(g == 1))

            def xs_of(c, kt):
                g, j = divmod(kt, XG)
                return x_tiles[(c, g)][:, j * SC:(j + 1) * SC]

            def drain(c, ot, ps):
                orows = min(P, O_SHARD - ot * P)
                out_t = out_pool.tile([P, SC], mybir.dt.float32)
                nc.vector.tensor_scalar(
                    out=out_t[:orows, :],
                    in0=ps[:orows, :],
                    scalar1=scale_t[:orows, ot:ot + 1],
                    scalar2=bias_t[:orows, ot:ot + 1],
                    op0=mybir.AluOpType.mult,
                    op1=mybir.AluOpType.add,
                )
                nc.sync.dma_start(
                    yt[c, ot * P:ot * P + orows, :], out_t[:orows, :]
                )

            # ---- chunk 0: kt-outer over PSUM groups [8, 3] ----
            # The 8-wide group amortizes each just-arrived w/x k-slice over
            # 8 matmuls, keeping PE demand under the DMA delivery rate.
            for g0, g1 in ((0, 8), (8, OT)):
                psums = {}
                for ot in range(g0, g1):
                    psums[ot] = psum_pool.tile([P, SC], mybir.dt.float32,
                                               name=f"ps0_{ot}", tag="psum")
                for kt in range(KT):
                    xs = xs_of(0, kt)
                    for ot in range(g0, g1):
                        orows = min(P, O_SHARD - ot * P)
                        nc.tensor.matmul(
                            psums[ot][:orows, :],
                            w_res[kt][:, ot * P:ot * P + orows], xs,
                            start=(kt == 0), stop=(kt == KT - 1),
                        )
                if g0 == 0:
                    # prefetch chunk-1 x during the long first sweep, held
                    # behind the last weight DMA
                    for g in range(NXG):
                        emit_xg(1, g, after_w=KT - 1)
                for ot in range(g0, g1):
                    drain(0, ot, psums[ot])

            # ---- chunks 1..3: ot-outer (x prefetched, drains spread) ----
            for c in range(1, N_CHUNKS):
                for ot in range(OT):
                    if c + 1 < N_CHUNKS and ot < NXG:
                        emit_xg(c + 1, ot)
                    orows = min(P, O_SHARD - ot * P)
                    ps = psum_pool.tile([P, SC], mybir.dt.float32,
                                        name=f"ps{c}_{ot}", tag="psum")
                    for kt in range(KT):
                        nc.tensor.matmul(
                            ps[:orows, :],
                            w_res[kt][:, ot * P:ot * P + orows], xs_of(c, kt),
                            start=(kt == 0), stop=(kt == KT - 1),
                        )
                    drain(c, ot, ps)

    nc.compile()
    return nc


_NC_CACHE = None


def _get_nc():
    global _NC_CACHE
    if _NC_CACHE is None:
        _NC_CACHE = build_bass()
    return _NC_CACHE


def _prep_x(x):
    # [S, I] f32 -> xt [N_CHUNKS, NXG, P, XG*SC] f16 with
    # xt[c, g, p, j*SC + t] = xT[(g*XG + j)*P + p, c*SC + t]
    x2d = np.asarray(x).reshape(S, I).astype(np.float16)
    xT = np.ascontiguousarray(x2d.T)                        # [I, S]
    v = xT.reshape(NXG, XG, P, N_CHUNKS, SC)
    return np.ascontiguousarray(v.transpose(3, 0, 2, 1, 4)).reshape(
        N_CHUNKS, NXG, P, XG * SC)


def run(inputs, trace=False, trace_cores=None, tmpdir=None):
    x = np.asarray(inputs["x"])
    w = np.asarray(inputs["weight_int8"])
    scale = np.asarray(inputs["scale"], dtype=np.float32)
    bias = np.asarray(inputs["bias"], dtype=np.float32)

    xt = _prep_x(x)
    w16 = w.astype(np.float16)                              # int8 exact in fp16

    def col_pt(v):
        # [1376] -> [128, 11] with [p, t] = v[t*128 + p]; tail padded with 0
        tmp = np.zeros(OT * P, dtype=np.float32)
        tmp[:O_SHARD] = v
        return np.ascontiguousarray(tmp.reshape(OT, P).T)

    in_maps = []
    for c in range(N_CORES):
        sl = slice(c * O_SHARD, (c + 1) * O_SHARD)
        wtc = np.ascontiguousarray(w16[sl, :].T).reshape(KT, P, O_SHARD)
        in_maps.append({
            "xt": xt,
            "wt": wtc,
            "scale": col_pt(scale[sl]),
            "bias": col_pt(bias[sl]),
        })

    nc = _get_nc()
    kwargs = {}
    if trace:
        kwargs["trace"] = True
        if trace_cores is not None:
            kwargs["trace_cores"] = trace_cores
        if tmpdir is not None:
            kwargs["tmpdir"] = tmpdir
    res = run_bass_kernel_spmd(nc, in_maps, core_ids=list(range(N_CORES)), **kwargs)

    # yt [4, 1376, 512] per core -> [1376, 2048]; stack cores along O.
    parts = [
        np.asarray(res.results[c]["yt"]).transpose(1, 0, 2).reshape(O_SHARD, S)
        for c in range(N_CORES)
    ]
    yt_full = np.concatenate(parts, axis=0)                 # [O, S]
    out = np.ascontiguousarray(yt_full.T).reshape(B, S, O).astype(
        np.float32, copy=False)
    if trace:
        return out, res
    return out


def kernel(**inputs) -> np.ndarray:
    return run(inputs, trace=False)


# revision 16
# speedup vs baseline: 1.0207x; 1.0207x over previous
"""CompressedLinear (int8 weight, per-row scale) on 8 Trainium2 NeuronCores.

Math: y[b,s,o] = sum_i x[b,s,i] * (w_int8[o,i] * scale[o]) + bias[o]

Strategy (tensor-parallel over out_features, per sharding hint):
  - O = 11008 = 86 o-tiles of 128 rows; S = 2048 = 4 s-chunks of 512.
    The 344 (o-tile, s-chunk) blocks are spread EXACTLY 43 per core
    (1376 matmuls of [128x128]@[128x512] each) — no 96-row padded tile,
    no per-core imbalance.  Each core owns 10 primary o-tiles (all 4
    chunks) plus an "A" tile (2 chunks) and a "B" tile (1 chunk); a
    per-core permutation of the x chunk slots makes the device program
    identical across cores (SPMD) while the leftover tiles' chunks tile
    exactly across cores.
  - Scale is applied to the matmul OUTPUT (algebraically identical), so
    the device matmuls run on raw int8 weights cast to fp16 (exact);
    x in fp16 bounds the output relative error at ~2e-4.
  - All dtype conversion happens on the HOST, so every device DMA is a
    plain same-dtype HWDGE transfer from a contiguous HBM block.
  - x slot 0 runs kt-outer over an 8-o-tile PSUM group (amortizes each
    just-arrived w/x k-slice over 8 matmuls, keeping PE demand under the
    shared DMA delivery rate), then the 3 remaining o-tiles.  Slots 1-3
    run ot-outer (x prefetched, drains spread, minimal PSUM pressure).
  - Per-partition affine (scale, bias) is fused into the PSUM eviction.
"""

import numpy as np

import concourse.bass as bass
import concourse.tile as tile
from concourse import bacc, mybir
from concourse.bass_utils import run_bass_kernel_spmd

B = 1
S = 2048
I = 4096
O = 11008
N_CORES = 8
P = 128
SC = 512                 # s-columns per matmul (one PSUM bank of fp32)
N_SLOTS = S // SC        # 4 x-chunk slots
KT = I // P              # 32 k-slices
XG = 4                   # k-slices per x DMA group
NXG = KT // XG           # 8 x groups per slot
NT = O // P              # 86 o-tiles globally
NW = 12                  # weight-tile slots per core (10 primary + A + B)

# Per-core x-slot permutation: slot j holds real s-chunk PI[c][j].
PI = [(0, 1, 2, 3), (2, 3, 0, 1), (0, 1, 3, 2), (2, 3, 1, 0),
      (0, 1, 2, 3), (2, 3, 0, 1), (0, 1, 3, 2), (2, 3, 1, 0)]
# Leftover o-tiles 80..85: per-core (A, B) weight-slot contents.
AB = [(80, 82), (80, 83), (81, 82), (81, 83),
      (82, 84), (83, 85), (84, 84), (85, 85)]
# w-slot lists per x slot: slot 0/1 include A (w=10), slot 2 includes
# B (w=11), slot 3 only primaries.
WL = [list(range(10)) + [10],
      list(range(10)) + [10],
      list(range(10)) + [11],
      list(range(10))]
N_BLOCKS = sum(len(w) for w in WL)  # 43


def _o_tile(c, w):
    if w < 10:
        return 10 * c + w
    return AB[c][w - 10]


def _check_cover():
    seen = set()
    for c in range(N_CORES):
        for j in range(N_SLOTS):
            for w in WL[j]:
                key = (_o_tile(c, w), PI[c][j])
                assert key not in seen, key
                seen.add(key)
    assert len(seen) == NT * N_SLOTS, len(seen)


_check_cover()


def build_bass():
    MM_DT = mybir.dt.float16
    nc = bacc.Bacc("TRN2", target_bir_lowering=False, debug=False)

    xt = nc.dram_tensor("xt", [N_SLOTS, NXG, P, XG * SC], MM_DT,
                        kind="ExternalInput").ap()
    wt = nc.dram_tensor("wt", [KT, P, NW * P], MM_DT,
                        kind="ExternalInput").ap()
    # scale/bias pre-rearranged on host to [p, w] per weight slot
    scale = nc.dram_tensor("scale", [P, NW], mybir.dt.float32,
                           kind="ExternalInput").ap()
    bias = nc.dram_tensor("bias", [P, NW], mybir.dt.float32,
                          kind="ExternalInput").ap()
    yt = nc.dram_tensor("yt", [N_BLOCKS, P, SC], mybir.dt.float32,
                        kind="ExternalOutput").ap()

    with tile.TileContext(nc) as tc:
        with (
            tc.tile_pool(name="wres", bufs=1) as wres_pool,
            tc.tile_pool(name="consts", bufs=1) as const_pool,
            tc.tile_pool(name="xpool", bufs=16) as xpool,
            tc.tile_pool(name="outp", bufs=4) as out_pool,
            tc.tile_pool(name="psum", bufs=8, space="PSUM") as psum_pool,
        ):
            w_res = [None] * KT
            w_dmas = [None] * KT
            x_tiles = {}

            def emit_w(kt):
                w_kt = wres_pool.tile([P, NW * P], MM_DT, tag=f"w{kt}")
                w_dmas[kt] = nc.sync.dma_start(w_kt[:], wt[kt])
                w_res[kt] = w_kt

            def emit_xg(j, g, after_w=None, split=False):
                t = xpool.tile([P, XG * SC], MM_DT, tag="xg")
                if split:
                    # two half-DMAs: the first two k-slices land a transfer
                    # earlier, so the first real matmuls start sooner
                    h = XG * SC // 2
                    xds = [
                        nc.scalar.dma_start(t[:, :h], xt[j, g, :, :h]),
                        nc.scalar.dma_start(t[:, h:], xt[j, g, :, h:]),
                    ]
                else:
                    xds = [nc.scalar.dma_start(t[:], xt[j, g])]
                if after_w is not None:
                    # The per-core HBM/DMA bandwidth is shared across queues;
                    # hold prefetches back so the startup-critical weight
                    # stream is never starved.
                    for xd in xds:
                        bass._add_dep_helper(
                            xd.ins, w_dmas[after_w].ins, sync=True,
                            reason="pace x prefetch behind startup w stream",
                        )
                x_tiles[(j, g)] = t

            # First weight slice + first x group ride at the head of their
            # queues so the first real matmul's inputs land ASAP.
            emit_w(0)
            emit_xg(0, 0, split=True)

            # PE warm-up: dependency-free matmuls keep the PE busy during
            # the initial DMA window so the HAM clock gate opens (K=8/8)
            # before the first real matmul issues.  Sized so warm-up (at
            # half clock until the gate opens ~4us in) ends right as the
            # first w/x tiles land (~11.9us).
            warm_sb = const_pool.tile([P, P], MM_DT)
            nc.any.memset(warm_sb[:], 0.0)
            warm_ps = psum_pool.tile([P, P], mybir.dt.float32,
                                     name="warm_ps", tag="psum")
            N_WARM = 42
            for i in range(N_WARM):
                nc.tensor.matmul(
                    warm_ps[:], warm_sb[:], warm_sb[:],
                    start=(i == 0), stop=(i == N_WARM - 1),
                )

            # per-partition scale/bias columns, host-rearranged; gpsimd queue
            # keeps them entirely off the startup-critical sync/scalar queues
            # (not needed until the first PSUM drain).
            scale_t = const_pool.tile([P, NW], mybir.dt.float32)
            bias_t = const_pool.tile([P, NW], mybir.dt.float32)
            nc.gpsimd.dma_start(scale_t[:], scale[:, :])
            nc.gpsimd.dma_start(bias_t[:], bias[:, :])

            # Remaining weights (sync queue) and slot-0 x groups (scalar
            # queue).  x group g is consumed at kt=4g; pacing it behind
            # the w stream keeps the shared DMA bandwidth on the weights.
            for kt in range(1, KT):
                emit_w(kt)
            for g in range(1, NXG):
                emit_xg(0, g, after_w=max(1, 4 * g - 3))

            def xs_of(j, kt):
                g, r = divmod(kt, XG)
                return x_tiles[(j, g)][:, r * SC:(r + 1) * SC]

            block_id = {}
            bid = 0
            for j in range(N_SLOTS):
                for w in WL[j]:
                    block_id[(j, w)] = bid
                    bid += 1

            def drain(j, w, ps):
                out_t = out_pool.tile([P, SC], mybir.dt.float32)
                nc.vector.tensor_scalar(
                    out=out_t[:],
                    in0=ps[:],
                    scalar1=scale_t[:, w:w + 1],
                    scalar2=bias_t[:, w:w + 1],
                    op0=mybir.AluOpType.mult,
                    op1=mybir.AluOpType.add,
                )
                nc.sync.dma_start(yt[block_id[(j, w)]], out_t[:])

            # ---- x slot 0: kt-outer over PSUM groups [8, 3] ----
            wl0 = WL[0]
            for lo, hi in ((0, 8), (8, len(wl0))):
                psums = {}
                for w in wl0[lo:hi]:
                    psums[w] = psum_pool.tile([P, SC], mybir.dt.float32,
                                              name=f"ps0_{w}", tag="psum")
                for kt in range(KT):
                    xs = xs_of(0, kt)
                    for w in wl0[lo:hi]:
                        nc.tensor.matmul(
                            psums[w][:],
                            w_res[kt][:, w * P:(w + 1) * P], xs,
                            start=(kt == 0), stop=(kt == KT - 1),
                        )
                if lo == 0:
                    # prefetch slot-1 x during the long first sweep, held
                    # behind the last weight DMA
                    for g in range(NXG):
                        emit_xg(1, g, after_w=KT - 1)
                for w in wl0[lo:hi]:
                    drain(0, w, psums[w])

            # ---- x slots 1..3: ot-outer (x prefetched, drains spread) ----
            for j in range(1, N_SLOTS):
                for oi, w in enumerate(WL[j]):
                    if j + 1 < N_SLOTS and oi < NXG:
                        emit_xg(j + 1, oi)
                    ps = psum_pool.tile([P, SC], mybir.dt.float32,
                                        name=f"ps{j}_{w}", tag="psum")
                    for kt in range(KT):
                        nc.tensor.matmul(
                            ps[:],
                            w_res[kt][:, w * P:(w + 1) * P], xs_of(j, kt),
                            start=(kt == 0), stop=(kt == KT - 1),
                        )
                    drain(j, w, ps)

    nc.compile()
    return nc


_NC_CACHE = None


def _get_nc():
    global _NC_CACHE
    if _NC_CACHE is None:
        _NC_CACHE = build_bass()
    return _NC_CACHE


def _prep_x(x):
    # [S, I] f32 -> [N_SLOTS(real chunk), NXG, P, XG*SC] f16 with
    # [c, g, p, r*SC + t] = xT[(g*XG + r)*P + p, c*SC + t]
    x2d = np.asarray(x).reshape(S, I).astype(np.float16)
    xT = np.ascontiguousarray(x2d.T)                        # [I, S]
    v = xT.reshape(NXG, XG, P, N_SLOTS, SC)
    return np.ascontiguousarray(v.transpose(3, 0, 2, 1, 4)).reshape(
        N_SLOTS, NXG, P, XG * SC)


def run(inputs, trace=False, trace_cores=None, tmpdir=None):
    x = np.asarray(inputs["x"])
    w = np.asarray(inputs["weight_int8"])
    scale = np.asarray(inputs["scale"], dtype=np.float32)
    bias = np.asarray(inputs["bias"], dtype=np.float32)

    xt_nat = _prep_x(x)                                     # natural chunks
    w16 = w.astype(np.float16)                              # int8 exact in fp16

    in_maps = []
    for c in range(N_CORES):
        tiles = [_o_tile(c, w_) for w_ in range(NW)]
        rows = np.concatenate([w16[t * P:(t + 1) * P, :] for t in tiles])
        wtc = np.ascontiguousarray(rows.T).reshape(KT, P, NW * P)
        sc_c = np.stack([scale[t * P:(t + 1) * P] for t in tiles], axis=1)
        bi_c = np.stack([bias[t * P:(t + 1) * P] for t in tiles], axis=1)
        in_maps.append({
            "xt": np.ascontiguousarray(xt_nat[list(PI[c])]),
            "wt": wtc,
            "scale": np.ascontiguousarray(sc_c.astype(np.float32)),
            "bias": np.ascontiguousarray(bi_c.astype(np.float32)),
        })

    nc = _get_nc()
    kwargs = {}
    if trace:
        kwargs["trace"] = True
        if trace_cores is not None:
            kwargs["trace_cores"] = trace_cores
        if tmpdir is not None:
            kwargs["tmpdir"] = tmpdir
    res = run_bass_kernel_spmd(nc, in_maps, core_ids=list(range(N_CORES)), **kwargs)

    out2d = np.empty((O, S), dtype=np.float32)
    for c in range(N_CORES):
        ytc = np.asarray(res.results[c]["yt"])              # [43, P, SC]
        bid = 0
        for j in range(N_SLOTS):
            ch = PI[c][j]
            for w_ in WL[j]:
                t = _o_tile(c, w_)
                out2d[t * P:(t + 1) * P, ch * SC:(ch + 1) * SC] = ytc[bid]
                bid += 1
    out = np.ascontiguousarray(out2d.T).reshape(B, S, O)
    if trace:
        return out, res
    return out


def kernel(**inputs) -> np.ndarray:
    return run(inputs, trace=False)
